# revision 58
# baseline (speedup 1.0000x reference)
"""Trainium2 Bass kernel for multi-head causal attention with rotary embeddings.

Problem shapes (hardcoded):
  hidden_states [2, 2048, 1024] f32, W_qkv [1024, 3072], W_out [1024, 1024],
  b_out [1024], is_causal scalar. 16 heads x 64 dim, rope theta 10000.

Sharding over 8 cores: core c -> batch c//4, heads 4*(c%4) .. 4*(c%4)+3
(data parallel over batch x tensor parallel over heads; W_qkv column-parallel,
W_out row-parallel; per-core partial outputs are summed on host).

Rope trick: head-dim columns of Wq/Wk are de-interleaved on the host
(pairs (2i, 2i+1) -> (i, i+32)) so on-chip rope is a contiguous half-swap;
scores are invariant because q and k share the permutation.

Schedule design (v4, tuned against NTFF hardware profiles; ~180us/NEFF
fast-power-mode vs 308us for the previous filler-based design):
  - One linear emission stream: proj rb0 | attn(qb,p0) | proj rb_{qb+1} |
    attn(qb,p1) | outproj qb | ... The Tile greedy scheduler backfills PE
    idle slots in ACT-bound attention stretches with the next projection
    block's matmuls (no hand-rolled filler machinery).
  - The 1/8 score scale is folded into the exp activation's scale parameter
    (free on ACT) instead of a separate scaled psum evacuation.
  - Softmax denominators ride as an appended ones-row of V (PV row 64);
    reciprocal uses reciprocal_approx_fast (~5x cheaper than exact; input
    must sit at partition 0 - the custom op misreads other bases),
    partition-broadcast on gpsimd, normalize multiplies on DVE, the whole
    chain at high priority (it gates pv-slot reuse and outproj).
  - The score h2 pairs run concurrently on the PE (row groups 0-1 / 2-3
    via base-partition-derived tile_position); projections are emitted in
    2-tile passes so the shared 2-bank psum pool ping-pongs.
  - q/k evac + rope run at high priority (q first: the NEW q gates the
    next query block's first score matmuls): the ACT-copy -> DVE-rope chain
    otherwise queues behind diagonal-chunk masks and stalls the next query
    block's score matmuls.
  - PSUM budget: scores 2x[128,1024] (4 banks) + PV 2x[65,512] (2 banks) +
    shared proj/outproj pool 2x[128,512] (2 banks) = 8 banks exactly. The
    last block's outproj borrows freed score banks; its normalize takes a
    latency path (denominator read from pv psum, broadcast as two rank-1
    PE matmuls into freed pv banks).
  - HAM warmup: 72 throwaway matmuls cover the initial DMA ramp and the
    final normalize gap so real work runs at the 2.4 GHz (K=8/8) clock;
    outproj(1) is parked as inline fillers between the last pair-block's
    chunks to keep PE duty (and the clock) up through the exp-paced tail.
  - PV lags scores by 2 chunks; the multiplicative 0/1 causal mask runs on
    DVE after exp as one [128,2,128] op against a doubled tri table.
  - fp8/DoubleRow was tried and reverted: e4m3 quantization of h/W alone
    costs 6.6% rel error (random-sign sums do not average it out) vs the
    2e-2 gate.
"""

import numpy as np

B, N, D = 2, 2048, 1024
H, DH = 16, 64
THETA = 10000.0
NCORES = 8
KC = D // 128        # 8 contraction chunks for the projections
NKC = N // 128       # 16 key chunks
NQB = N // 512       # 4 query blocks
NRB = N // 512       # 4 row blocks for projections

_compiled = {}


def _build_nc(causal: bool, reps: int = 1):
    import concourse.bass as bass
    import concourse.tile as tile
    from concourse import bacc, mybir

    f32 = mybir.dt.float32
    bf16 = mybir.dt.bfloat16
    Exp = mybir.ActivationFunctionType.Exp

    nc = bacc.Bacc("TRN2", target_bir_lowering=False)
    hT_d = nc.dram_tensor("hidden_T", [D, N], bf16, kind="ExternalInput")
    w_d = nc.dram_tensor("w_all", [D, 768], bf16, kind="ExternalInput")
    wout_d = nc.dram_tensor("w_out", [256, 1024], bf16, kind="ExternalInput")
    cos_d = nc.dram_tensor("cos_t", [128, N], bf16, kind="ExternalInput")
    sin_d = nc.dram_tensor("sin_t", [128, N], bf16, kind="ExternalInput")
    tri_d = nc.dram_tensor("tri", [128, 256], bf16, kind="ExternalInput")
    out_d = nc.dram_tensor("out_partial", [N, 1024], bf16, kind="ExternalOutput")

    w_r = w_d.rearrange("(a p) c -> p a c", p=128)
    wout_r = wout_d.rearrange("(a p) c -> p a c", p=128)

    with tile.TileContext(nc) as tc:
        with (
            tc.tile_pool(name="consts", bufs=1) as consts,
            tc.tile_pool(name="qk", bufs=1) as qkp,
            tc.tile_pool(name="vaugp", bufs=1) as vaugp,
            tc.tile_pool(name="atp", bufs=1) as atp,
            tc.tile_pool(name="htp", bufs=1) as htp,
        ):
            w_sb = consts.tile([128, KC, 768], bf16, tag="w_sb", name="w_sb")
            wout_sb = consts.tile([128, 2, 1024], bf16, tag="wout_sb",
                                  name="wout_sb")
            cos_sb = consts.tile([128, N], bf16, tag="cos_sb", name="cos_sb")
            sin_sb = consts.tile([128, N], bf16, tag="sin_sb", name="sin_sb")
            tri_sb = consts.tile([128, 256], bf16, tag="tri_sb", name="tri_sb")

            ones_col = consts.tile([1, 64], bf16, tag="ones_col",
                                   name="ones_col")
            nc.vector.memset(ones_col, 1.0)
            dummy_sb = consts.tile([128, 128], bf16, tag="dummy_sb",
                                   name="dummy_sb")
            nc.vector.memset(dummy_sb, 0.0)

            # preload the Exp activation table while the first DMAs land
            warm_in = consts.tile([1, 1], f32, tag="warm_in", name="warm_in")
            warm_out = consts.tile([1, 1], f32, tag="warm_out", name="warm_out")
            nc.vector.memset(warm_in, 0.0)
            nc.scalar.activation(warm_out, warm_in, func=Exp)

            for rep in range(reps):
                qkT = {}
                for pair in range(2):
                    for qk in range(2):
                        for rb in range(NRB):
                            t = qkp.tile([128, 512], bf16,
                                         tag=f"qkT{pair}{qk}{rb}",
                                         name=f"qkT{pair}{qk}{rb}")
                            qkT[(pair, qk, rb)] = t
                vaug = vaugp.tile([128, NKC, 4, 65], bf16, tag="vaug",
                                  name="vaug")
                nc.vector.memset(vaug[:, :, :, 64:65], 1.0)
                A_T = {}
                for pair in range(2):
                    A_T[pair] = atp.tile([128, N], bf16, tag=f"AT{pair}",
                                         name=f"AT{pair}")

                htA = {}
                htB = {}
                for kc in range(KC):
                    htA[kc] = htp.tile([128, 1024], bf16, tag=f"htA_{kc}",
                                       name=f"htA_{kc}")
                    htB[kc] = htp.tile([128, 1024], bf16, tag=f"htB_{kc}",
                                       name=f"htB_{kc}")

                def ht_slice(rb, kc):
                    if rb < 2:
                        return htA[kc][:, rb * 512:(rb + 1) * 512]
                    return htB[kc][:, (rb - 2) * 512:(rb - 1) * 512]

                # ---- DMA plan ----
                for kc in range(KC):
                    if rep == 0:
                        nc.sync.dma_start(out=w_sb[:, kc, :], in_=w_r[:, kc, :])
                    nc.sync.dma_start(
                        out=htA[kc], in_=hT_d[kc * 128:(kc + 1) * 128, 0:1024])
                if rep == 0:
                    nc.sync.dma_start(out=cos_sb, in_=cos_d[:, :])
                    nc.sync.dma_start(out=sin_sb, in_=sin_d[:, :])
                    nc.sync.dma_start(out=tri_sb, in_=tri_d[:, :])
                for kc in range(KC):
                    nc.sync.dma_start(
                        out=htB[kc],
                        in_=hT_d[kc * 128:(kc + 1) * 128, 1024:2048])
                if rep == 0:
                    nc.sync.dma_start(out=wout_sb, in_=wout_r)

                with (
                    tc.tile_pool(name="stp", bufs=2, space="PSUM") as stp,
                    tc.tile_pool(name="pvp", bufs=2, space="PSUM") as pvp,
                    tc.tile_pool(name="genp", bufs=2, space="PSUM") as genp,
                    tc.tile_pool(name="psbp", bufs=8) as psbp,
                    tc.tile_pool(name="pvcp", bufs=2) as pvcp,
                    tc.tile_pool(name="smallp", bufs=4) as smallp,
                    tc.tile_pool(name="ropep", bufs=2) as ropep,
                    tc.tile_pool(name="obufp", bufs=4) as obufp,
                ):
                    # HAM warmup: the PE is otherwise idle until the first
                    # input DMAs land (~6us), which guarantees a cold-clock
                    # (K=4/8) start. Throwaway matmuls on a zero tile flip
                    # the activity window to full clock before real work.
                    if rep == 0:
                        dmt = genp.tile([128, 512], f32, tag="g",
                                        name="warm_mm")
                        for _ in range(72):
                            nc.tensor.matmul(dmt[:, 0:128], lhsT=dummy_sb,
                                             rhs=dummy_sb, start=True,
                                             stop=True)
                    def rope_tile(pair, qk, rb):
                        cs = slice(rb * 512, (rb + 1) * 512)
                        t = qkT[(pair, qk, rb)]
                        tmp = ropep.tile([128, 512], bf16, tag="ropetmp",
                                         name="ropetmp")
                        for b0 in (0, 64):
                            nc.vector.tensor_mul(
                                tmp[b0:b0 + 32, :], t[b0 + 32:b0 + 64, :],
                                sin_sb[b0 + 32:b0 + 64, cs])
                            nc.vector.tensor_mul(
                                tmp[b0 + 32:b0 + 64, :], t[b0:b0 + 32, :],
                                sin_sb[b0:b0 + 32, cs])
                        nc.vector.tensor_mul(t[:, :], t[:, :], cos_sb[:, cs])
                        nc.vector.tensor_add(t[:, :], t[:, :], tmp)

                    def proj_pass_qk(rb, pair):
                        ps = []
                        for qk in range(2):
                            ps.append(genp.tile([128, 512], f32, tag="g",
                                                name=f"qk_ps{pair}{qk}"))
                        for kc in range(KC):
                            ht = ht_slice(rb, kc)
                            for qk in range(2):
                                col0 = (pair * 2 + qk) * 128
                                nc.tensor.matmul(
                                    ps[qk],
                                    lhsT=w_sb[:, kc, col0:col0 + 128],
                                    rhs=ht,
                                    start=(kc == 0), stop=(kc == KC - 1))
                        # the NEW q gates the next query block's first score
                        # matmuls (kc 0 uses the oldest k), so q's evac +
                        # rope jump ahead of nearby masks on the ACT/DVE
                        # queues; relative offset (not absolute 0) so the
                        # normalize chain still outranks them at block ends
                        with tc.high_priority():
                            nc.scalar.copy(qkT[(pair, 0, rb)], ps[0])
                            rope_tile(pair, 0, rb)
                        with tc.high_priority():
                            nc.scalar.copy(qkT[(pair, 1, rb)], ps[1])
                            rope_tile(pair, 1, rb)

                    def proj_pass_v(rb, half):
                        rcs = [rb * 4 + half * 2, rb * 4 + half * 2 + 1]
                        ps = []
                        for j in range(2):
                            ps.append(genp.tile([128, 512], f32, tag="g",
                                                name=f"v_ps{j}"))
                        for kc in range(KC):
                            for j, rc in enumerate(rcs):
                                nc.tensor.matmul(
                                    ps[j][:, 0:256],
                                    lhsT=ht_slice(rb, kc)[
                                        :, (rc % 4) * 128:(rc % 4 + 1) * 128],
                                    rhs=w_sb[:, kc, 512:768],
                                    start=(kc == 0), stop=(kc == KC - 1))
                        for j, rc in enumerate(rcs):
                            nc.scalar.copy(
                                vaug[:, rc, :, 0:64],
                                ps[j][:, 0:256].rearrange(
                                    "p (a b) -> p a b", a=4))

                    def proj_rb(rb):
                        proj_pass_qk(rb, 0)
                        proj_pass_qk(rb, 1)
                        proj_pass_v(rb, 0)
                        proj_pass_v(rb, 1)

                    LAG = 2

                    def attn_block(qb, pair, fillers=None):
                        kmax = 4 * qb + 3 if causal else NKC - 1
                        pvs = []
                        for h2 in range(2):
                            pvs.append(pvp.tile([65, 512], f32, tag="pv",
                                                name=f"pv{h2}"))
                        qT = qkT[(pair, 0, qb)]

                        def emit_pv(ent):
                            kc, qlo, psb = ent
                            for h2 in range(2):
                                nc.tensor.matmul(
                                    pvs[h2][:, qlo:],
                                    lhsT=vaug[:, kc, pair * 2 + h2, :],
                                    rhs=psb[:, h2 * 512 + qlo:(h2 + 1) * 512],
                                    start=(kc == 0), stop=(kc == kmax))

                        pending = []
                        for kc in range(kmax + 1):
                            kT = qkT[(pair, 1, kc // 4)]
                            kc4 = kc % 4
                            qlo = (max(0, kc * 128 - qb * 512)
                                   if causal else 0)
                            st = stp.tile([128, 1024], f32, tag="st",
                                          name="st")
                            psb = psbp.tile([128, 1024], bf16, tag="psb",
                                            name="psb")
                            for h2 in range(2):
                                b0 = h2 * 64
                                nc.tensor.matmul(
                                    st[:, h2 * 512 + qlo:(h2 + 1) * 512],
                                    lhsT=kT[b0:b0 + 64,
                                            kc4 * 128:(kc4 + 1) * 128],
                                    rhs=qT[b0:b0 + 64, qlo:512],
                                    start=True, stop=True)
                            if qlo == 0:
                                nc.scalar.activation(psb[:, :], st[:, :],
                                                     func=Exp, scale=0.125)
                            else:
                                st3 = st.rearrange("p (h q) -> p h q", h=2)
                                psb3 = psb.rearrange("p (h q) -> p h q", h=2)
                                nc.scalar.activation(
                                    psb3[:, :, qlo:], st3[:, :, qlo:],
                                    func=Exp, scale=0.125)
                            if causal and kc >= 4 * qb:
                                # gpsimd: slower per-op than DVE but idle -
                                # the DVE queue at block boundaries is full
                                # of rope ops and starves the PV matmuls
                                psbm = psb.rearrange(
                                    "p (h q) -> p h q", h=2)[:, :,
                                                            qlo:qlo + 128]
                                nc.gpsimd.tensor_mul(
                                    psbm, psbm,
                                    tri_sb.rearrange("p (h q) -> p h q", h=2))
                            pending.append((kc, qlo, psb))
                            if len(pending) > LAG:
                                emit_pv(pending.pop(0))
                            if fillers and kc % 2 == 0:
                                fillers.pop(0)()
                        while pending:
                            emit_pv(pending.pop(0))

                        # normalize: evacuate pv psum fast, fast-approx
                        # reciprocal of the ones-row, broadcast on gpsimd,
                        # multiply into A_T on DVE. High priority: the pv
                        # slot release and A_T availability gate the next
                        # block's PV matmuls and the outproj, and these ops
                        # otherwise queue behind the diagonal-chunk masks.
                        # The LAST block takes a latency-optimized path: the
                        # denominator is read straight from pv psum, the
                        # broadcast runs as two rank-1 matmuls on the (idle)
                        # PE into the freed pv banks, and the pvc copies
                        # overlap the reciprocal.
                        last = (qb == NQB - 1 and pair == 1)
                        with tc.high_priority():
                            den = smallp.tile([1, 1024], f32, tag="den",
                                              name="den")
                            if last:
                                for h2 in range(2):
                                    nc.vector.tensor_copy(
                                        den[:, h2 * 512:(h2 + 1) * 512],
                                        pvs[h2][64:65, :])
                            rinv = smallp.tile([1, 1024], f32, tag="rinv",
                                               name="rinv")
                            if last:
                                # keep the PE clock warm across the final
                                # normalize chain: these become ready only
                                # once `den` is written (i.e. in the gap)
                                dmt2 = genp.tile([128, 512], f32, tag="g",
                                                 name="warm_mm2")
                                for _ in range(6):
                                    nc.tensor.matmul(
                                        dmt2, lhsT=den[0:1, 0:128],
                                        rhs=den[0:1, 0:512], start=True,
                                        stop=True)
                                nc.vector.reciprocal_approx_fast(rinv, den)
                                rinvb = smallp.tile([1, 1024], bf16,
                                                    tag="rinvb", name="rinvb")
                                nc.vector.tensor_copy(rinvb, rinv)
                            pvc = pvcp.tile([65, 1024], f32, tag="pvc",
                                            name="pvc")
                            for h2 in range(2):
                                nc.vector.tensor_copy(
                                    pvc[:, h2 * 512:(h2 + 1) * 512], pvs[h2])
                            if not last:
                                nc.vector.tensor_copy(den, pvc[64:65, :])
                                nc.vector.reciprocal_approx_fast(rinv, den)
                            if last:
                                bc_ps = []
                                for h2 in range(2):
                                    bp = pvp.tile([64, 512], f32, tag="pv",
                                                  name=f"bcps{h2}")
                                    nc.tensor.matmul(
                                        bp, lhsT=ones_col,
                                        rhs=rinvb[:,
                                                  h2 * 512:(h2 + 1) * 512],
                                        start=True, stop=True)
                                    bc_ps.append(bp)
                                for h2 in range(2):
                                    nc.vector.tensor_mul(
                                        A_T[pair][h2 * 64:(h2 + 1) * 64,
                                                  qb * 512:(qb + 1) * 512],
                                        pvc[0:64, h2 * 512:(h2 + 1) * 512],
                                        bc_ps[h2])
                            else:
                                bcast = smallp.tile([64, 1024], f32, tag="bc",
                                                    name="bc")
                                nc.gpsimd.partition_broadcast(bcast, rinv)
                                for h2 in range(2):
                                    nc.vector.tensor_mul(
                                        A_T[pair][h2 * 64:(h2 + 1) * 64,
                                                  qb * 512:(qb + 1) * 512],
                                        pvc[0:64, h2 * 512:(h2 + 1) * 512],
                                        bcast[:, h2 * 512:(h2 + 1) * 512])

                    def outproj(qb):
                        # for the last block, borrow the (now free) score
                        # psum banks so the pair-0 matmuls of several row
                        # chunks can pre-run while the final normalize chain
                        # still computes pair 1's A_T
                        last = (qb == NQB - 1)
                        for rc in range(4 * qb, 4 * qb + 4):
                            ob = obufp.tile([128, 1024], bf16, tag="ob",
                                            name="ob")
                            if last and rc % 4 < 2:
                                opw = stp.tile([128, 1024], f32, tag="st",
                                               name="opw")
                                halves = [opw[:, 0:512], opw[:, 512:1024]]
                            else:
                                halves = None
                            for half in range(2):
                                if halves is None:
                                    op = genp.tile([128, 512], f32, tag="g",
                                                   name="op")
                                else:
                                    op = halves[half]
                                for pair in range(2):
                                    nc.tensor.matmul(
                                        op,
                                        lhsT=A_T[pair][:,
                                                       rc * 128:(rc + 1) * 128],
                                        rhs=wout_sb[:, pair,
                                                    half * 512:(half + 1) * 512],
                                        start=(pair == 0), stop=(pair == 1))
                                if (rc + half) % 2 == 0:
                                    nc.scalar.copy(
                                        ob[:, half * 512:(half + 1) * 512], op)
                                else:
                                    nc.vector.tensor_copy(
                                        ob[:, half * 512:(half + 1) * 512], op)
                            nc.sync.dma_start(
                                out=out_d[rc * 128:(rc + 1) * 128, :], in_=ob)

                    # ---- main emission stream ----
                    ob_map = {}

                    def op_filler(rc, half):
                        def f():
                            if half == 0:
                                ob_map[rc] = obufp.tile(
                                    [128, 1024], bf16, tag="ob", name="ob")
                            ob = ob_map[rc]
                            op = genp.tile([128, 512], f32, tag="g",
                                           name="op")
                            for pair in range(2):
                                nc.tensor.matmul(
                                    op,
                                    lhsT=A_T[pair][:,
                                                   rc * 128:(rc + 1) * 128],
                                    rhs=wout_sb[:, pair,
                                                half * 512:(half + 1) * 512],
                                    start=(pair == 0), stop=(pair == 1))
                            if (rc + half) % 2 == 0:
                                nc.scalar.copy(
                                    ob[:, half * 512:(half + 1) * 512], op)
                            else:
                                nc.vector.tensor_copy(
                                    ob[:, half * 512:(half + 1) * 512], op)
                            if half == 1:
                                nc.sync.dma_start(
                                    out=out_d[rc * 128:(rc + 1) * 128, :],
                                    in_=ob_map.pop(rc))
                        return f

                    proj_rb(0)
                    attn_block(0, 0)
                    proj_rb(1)
                    attn_block(0, 1)
                    outproj(0)
                    attn_block(1, 0)
                    proj_rb(2)
                    attn_block(1, 1)
                    attn_block(2, 0)
                    proj_rb(3)
                    attn_block(2, 1)
                    attn_block(3, 0)
                    outproj(2)
                    # outproj(1) is parked as inline fillers between the
                    # last pair-block's chunks: real PE work that keeps the
                    # HAM clock up through the exp-paced final stretch
                    attn_block(3, 1, fillers=[
                        op_filler(rc, h) for rc in range(4, 8)
                        for h in range(2)])
                    # psum-free HAM filler: standalone dummy weight loads
                    # keep the PE array active through exp-paced stretches
                    # (every real matmul self-loads its weights, so these
                    # cannot corrupt results)
                    for _ in range(60):
                        nc.tensor.ldweights(dummy_sb)
                    outproj(3)
                    for _ in range(40):
                        nc.tensor.ldweights(dummy_sb)

    nc.compile()
    return nc


def _host_inputs(hidden_states, W_qkv, W_out):
    """Build the 8 per-core input maps."""
    import ml_dtypes
    bf16 = ml_dtypes.bfloat16
    hidden = np.ascontiguousarray(hidden_states, dtype=np.float32)
    W_qkv = np.asarray(W_qkv, dtype=np.float32)
    W_out = np.asarray(W_out, dtype=np.float32)
    Wq, Wk, Wv = W_qkv[:, :1024], W_qkv[:, 1024:2048], W_qkv[:, 2048:]

    perm = np.concatenate([np.arange(0, 64, 2), np.arange(1, 64, 2)])

    invf = THETA ** (-np.arange(0, 32, dtype=np.float64) * 2.0 / 64.0)
    ang = np.arange(N, dtype=np.float64)[:, None] * invf[None, :]  # [N, 32]
    jj = np.arange(64)
    cos64 = np.cos(ang)[:, jj % 32].T
    sin64 = np.sin(ang)[:, jj % 32].T
    # row r holds the sin factor applied when row r is the SOURCE of the
    # half-swap: rows j<32 feed dst j+32 with +sin; rows j>=32 feed dst j-32
    # with -sin.
    sgn = np.where(jj < 32, 1.0, -1.0)[:, None]
    cos_t = np.ascontiguousarray(np.tile(cos64, (2, 1)), dtype=bf16)
    sin_t = np.ascontiguousarray(np.tile(sgn * sin64, (2, 1)), dtype=bf16)
    # multiplicative mask: 1 where q >= k (valid), 0 where masked
    tri1 = np.where(
        np.arange(128)[None, :] >= np.arange(128)[:, None], 1.0, 0.0)
    tri = np.ascontiguousarray(
        np.concatenate([tri1, tri1], axis=1), dtype=bf16)

    hT = [np.ascontiguousarray(hidden[b].T.astype(bf16)) for b in range(B)]

    in_maps = []
    for c in range(NCORES):
        bb = c // 4
        bh = (c % 4) * 4

        def qk_cols(pair, qk):
            W = Wq if qk == 0 else Wk
            cols = []
            for h2 in range(2):
                hh = bh + pair * 2 + h2
                cols.extend(hh * 64 + perm)
            return W[:, np.array(cols)]

        w_all = np.ascontiguousarray(np.concatenate(
            [qk_cols(0, 0), qk_cols(0, 1), qk_cols(1, 0), qk_cols(1, 1),
             Wv[:, bh * 64:(bh + 4) * 64]], axis=1), dtype=bf16)
        wout_c = np.ascontiguousarray(W_out[bh * 64:(bh + 4) * 64, :], dtype=bf16)
        in_maps.append({
            "hidden_T": hT[bb],
            "w_all": w_all,
            "w_out": wout_c,
            "cos_t": cos_t,
            "sin_t": sin_t,
            "tri": tri,
        })
    return in_maps


def _unshard(results, b_out):
    out = np.zeros((B, N, 1024), dtype=np.float32)
    for c in range(NCORES):
        out[c // 4] += np.asarray(results[c]["out_partial"], dtype=np.float32)
    out += np.asarray(b_out, dtype=np.float32)[None, None, :]
    return out


def run(hidden_states, W_qkv, W_out, b_out, is_causal, **_ignored):
    causal = bool(int(np.asarray(is_causal)))
    key = ("nc", causal, 1)
    if key not in _compiled:
        _compiled[key] = _build_nc(causal)
    nc = _compiled[key]

    from concourse import bass2jax
    in_maps = _host_inputs(hidden_states, W_qkv, W_out)
    results = bass2jax.run_bass_via_pjrt(nc, in_maps, n_cores=NCORES)
    return _unshard(results, b_out), None


def profile_exec(hidden_states, W_qkv, W_out, b_out, is_causal,
                 neff_dir="/tmp/kprof", all_cores=False):
    """Run once under the axon NTFF profiling hook; return
    (output, exec_time_ns, trace_paths). exec_time_ns is the max profiled
    per-core NEFF execution span (neuron-profile ground truth)."""
    import contextlib
    import ctypes
    import os
    import shutil
    import sys as _sys

    causal = bool(int(np.asarray(is_causal)))
    key = ("nc", causal, 1)
    if key not in _compiled:
        _compiled[key] = _build_nc(causal)
    nc = _compiled[key]
    in_maps = _host_inputs(hidden_states, W_qkv, W_out)

    @contextlib.contextmanager
    def ntff_profile(output_dir, device_ids):
        import jax
        lib = ctypes.CDLL("/opt/axon/libaxon_pjrt.so")
        lib.axon_start_nrt_profile.argtypes = [
            ctypes.POINTER(ctypes.c_int64), ctypes.c_size_t]
        lib.axon_start_nrt_profile.restype = ctypes.c_int64
        lib.axon_stop_nrt_profile.argtypes = [ctypes.c_char_p]
        lib.axon_stop_nrt_profile.restype = ctypes.c_int64
        jax.devices()
        ids = (ctypes.c_int64 * len(device_ids))(*device_ids)
        rc = lib.axon_start_nrt_profile(ids, len(device_ids))
        if rc != 0:
            raise RuntimeError(f"axon_start_nrt_profile rc={rc}")
        try:
            yield
        finally:
            n = lib.axon_stop_nrt_profile(str(output_dir).encode())
            print(f"profile: {n} file(s) written to {output_dir}",
                  file=_sys.stderr)

    shutil.rmtree(neff_dir, ignore_errors=True)
    os.makedirs(neff_dir, exist_ok=True)
    from concourse import bass2jax
    model_indices = list(range(NCORES)) if all_cores else [0]
    with ntff_profile(neff_dir, model_indices):
        results = bass2jax.run_bass_via_pjrt(nc, in_maps, n_cores=NCORES)

    import gauge.profiler
    from concourse._compat import FishPath
    profile = gauge.profiler.Profile(
        profile_path=FishPath(neff_dir),
        kernel_dev_mode=True,
        profile_on_exit=False,
        bass_kernel=nc.m,
        offline_processing=True,
        fname="*_body*",
    )
    perfetto_results = profile.to_perfetto(model_index=tuple(model_indices))
    exec_ns = max(pr.exec_time_ns for pr in perfetto_results)
    traces = [pr.trace_path for pr in perfetto_results]
    return _unshard(results, b_out), exec_ns, traces


def kernel(hidden_states, W_qkv, W_out, b_out, is_causal):
    out, _ = run(hidden_states, W_qkv, W_out, b_out, is_causal)
    return out


# revision 59
# speedup vs baseline: 1.2301x; 1.2301x over previous
"""Trainium2 Bass kernel for multi-head causal attention with rotary embeddings.

Problem shapes (hardcoded):
  hidden_states [2, 2048, 1024] f32, W_qkv [1024, 3072], W_out [1024, 1024],
  b_out [1024], is_causal scalar. 16 heads x 64 dim, rope theta 10000.

Sharding over 8 cores: core c -> batch c//4, heads 4*(c%4) .. 4*(c%4)+3
(data parallel over batch x tensor parallel over heads; W_qkv column-parallel,
W_out row-parallel; per-core partial outputs are summed on host).

Rope trick: head-dim columns of Wq/Wk are de-interleaved on the host
(pairs (2i, 2i+1) -> (i, i+32)) so on-chip rope is a contiguous half-swap;
scores are invariant because q and k share the permutation.

Schedule design (v4, tuned against NTFF hardware profiles; ~180us/NEFF
fast-power-mode vs 308us for the previous filler-based design):
  - One linear emission stream: proj rb0 | attn(qb,p0) | proj rb_{qb+1} |
    attn(qb,p1) | outproj qb | ... The Tile greedy scheduler backfills PE
    idle slots in ACT-bound attention stretches with the next projection
    block's matmuls (no hand-rolled filler machinery).
  - The 1/8 score scale is folded into the exp activation's scale parameter
    (free on ACT) instead of a separate scaled psum evacuation.
  - Softmax denominators ride as an appended ones-row of V (PV row 64);
    reciprocal uses reciprocal_approx_fast (~5x cheaper than exact; input
    must sit at partition 0 - the custom op misreads other bases),
    partition-broadcast on gpsimd, normalize multiplies on DVE, the whole
    chain at high priority (it gates pv-slot reuse and outproj).
  - The score h2 pairs run concurrently on the PE (row groups 0-1 / 2-3
    via base-partition-derived tile_position); projections are emitted in
    2-tile passes so the shared 2-bank psum pool ping-pongs.
  - q/k evac + rope run at high priority (q first: the NEW q gates the
    next query block's first score matmuls): the ACT-copy -> DVE-rope chain
    otherwise queues behind diagonal-chunk masks and stalls the next query
    block's score matmuls.
  - PSUM budget: scores 2x[128,1024] (4 banks) + PV 2x[65,512] (2 banks) +
    shared proj/outproj pool 2x[128,512] (2 banks) = 8 banks exactly. The
    last block's outproj borrows freed score banks; its normalize takes a
    latency path (denominator read from pv psum, broadcast as two rank-1
    PE matmuls into freed pv banks).
  - HAM warmup: 72 throwaway matmuls cover the initial DMA ramp and the
    final normalize gap so real work runs at the 2.4 GHz (K=8/8) clock;
    outproj(1) is parked as inline fillers between the last pair-block's
    chunks to keep PE duty (and the clock) up through the exp-paced tail.
  - PV lags scores by 2 chunks; the multiplicative 0/1 causal mask runs on
    DVE after exp as one [128,2,128] op against a doubled tri table.
  - fp8/DoubleRow was tried and reverted: e4m3 quantization of h/W alone
    costs 6.6% rel error (random-sign sums do not average it out) vs the
    2e-2 gate.
"""

import numpy as np

B, N, D = 2, 2048, 1024
H, DH = 16, 64
THETA = 10000.0
NCORES = 8
KC = D // 128        # 8 contraction chunks for the projections
NKC = N // 128       # 16 key chunks
NQB = N // 512       # 4 query blocks
NRB = N // 512       # 4 row blocks for projections

_compiled = {}


def _build_nc(causal: bool, reps: int = 1):
    import concourse.bass as bass
    import concourse.tile as tile
    from concourse import bacc, mybir

    f32 = mybir.dt.float32
    bf16 = mybir.dt.bfloat16
    Exp = mybir.ActivationFunctionType.Exp

    nc = bacc.Bacc("TRN2", target_bir_lowering=False)
    hT_d = nc.dram_tensor("hidden_T", [D, N], bf16, kind="ExternalInput")
    w_d = nc.dram_tensor("w_all", [D, 768], bf16, kind="ExternalInput")
    wout_d = nc.dram_tensor("w_out", [256, 1024], bf16, kind="ExternalInput")
    cos_d = nc.dram_tensor("cos_t", [128, N], bf16, kind="ExternalInput")
    sin_d = nc.dram_tensor("sin_t", [128, N], bf16, kind="ExternalInput")
    tri_d = nc.dram_tensor("tri", [128, 256], bf16, kind="ExternalInput")
    out_d = nc.dram_tensor("out_partial", [N, 1024], bf16, kind="ExternalOutput")

    w_r = w_d.rearrange("(a p) c -> p a c", p=128)
    wout_r = wout_d.rearrange("(a p) c -> p a c", p=128)

    with tile.TileContext(nc) as tc:
        with (
            tc.tile_pool(name="consts", bufs=1) as consts,
            tc.tile_pool(name="qk", bufs=1) as qkp,
            tc.tile_pool(name="vaugp", bufs=1) as vaugp,
            tc.tile_pool(name="atp", bufs=1) as atp,
            tc.tile_pool(name="htp", bufs=1) as htp,
        ):
            w_sb = consts.tile([128, KC, 768], bf16, tag="w_sb", name="w_sb")
            wout_sb = consts.tile([128, 2, 1024], bf16, tag="wout_sb",
                                  name="wout_sb")
            cos_sb = consts.tile([128, N], bf16, tag="cos_sb", name="cos_sb")
            sin_sb = consts.tile([128, N], bf16, tag="sin_sb", name="sin_sb")
            tri_sb = consts.tile([128, 256], bf16, tag="tri_sb", name="tri_sb")

            ones_col = consts.tile([1, 64], bf16, tag="ones_col",
                                   name="ones_col")
            nc.vector.memset(ones_col, 1.0)
            dummy_sb = consts.tile([128, 128], bf16, tag="dummy_sb",
                                   name="dummy_sb")
            nc.vector.memset(dummy_sb, 0.0)

            # preload the Exp activation table while the first DMAs land
            warm_in = consts.tile([1, 1], f32, tag="warm_in", name="warm_in")
            warm_out = consts.tile([1, 1], f32, tag="warm_out", name="warm_out")
            nc.vector.memset(warm_in, 0.0)
            nc.scalar.activation(warm_out, warm_in, func=Exp)

            for rep in range(reps):
                qkT = {}
                for pair in range(2):
                    for qk in range(2):
                        for rb in range(NRB):
                            t = qkp.tile([128, 512], bf16,
                                         tag=f"qkT{pair}{qk}{rb}",
                                         name=f"qkT{pair}{qk}{rb}")
                            qkT[(pair, qk, rb)] = t
                vaug = vaugp.tile([128, NKC, 4, 65], bf16, tag="vaug",
                                  name="vaug")
                nc.vector.memset(vaug[:, :, :, 64:65], 1.0)
                A_T = {}
                for pair in range(2):
                    A_T[pair] = atp.tile([128, N], bf16, tag=f"AT{pair}",
                                         name=f"AT{pair}")

                htA = {}
                htB = {}
                for kc in range(KC):
                    htA[kc] = htp.tile([128, 1024], bf16, tag=f"htA_{kc}",
                                       name=f"htA_{kc}")
                    htB[kc] = htp.tile([128, 1024], bf16, tag=f"htB_{kc}",
                                       name=f"htB_{kc}")

                def ht_slice(rb, kc):
                    if rb < 2:
                        return htA[kc][:, rb * 512:(rb + 1) * 512]
                    return htB[kc][:, (rb - 2) * 512:(rb - 1) * 512]

                # ---- DMA plan ----
                for kc in range(KC):
                    if rep == 0:
                        nc.sync.dma_start(out=w_sb[:, kc, :], in_=w_r[:, kc, :])
                    nc.sync.dma_start(
                        out=htA[kc], in_=hT_d[kc * 128:(kc + 1) * 128, 0:1024])
                if rep == 0:
                    nc.sync.dma_start(out=cos_sb, in_=cos_d[:, :])
                    nc.sync.dma_start(out=sin_sb, in_=sin_d[:, :])
                    nc.sync.dma_start(out=tri_sb, in_=tri_d[:, :])
                for kc in range(KC):
                    nc.sync.dma_start(
                        out=htB[kc],
                        in_=hT_d[kc * 128:(kc + 1) * 128, 1024:2048])
                if rep == 0:
                    nc.sync.dma_start(out=wout_sb, in_=wout_r)

                with (
                    tc.tile_pool(name="stp", bufs=2, space="PSUM") as stp,
                    tc.tile_pool(name="pvp", bufs=2, space="PSUM") as pvp,
                    tc.tile_pool(name="genp", bufs=2, space="PSUM") as genp,
                    tc.tile_pool(name="psbp", bufs=8) as psbp,
                    tc.tile_pool(name="pvcp", bufs=2) as pvcp,
                    tc.tile_pool(name="smallp", bufs=4) as smallp,
                    tc.tile_pool(name="ropep", bufs=2) as ropep,
                    tc.tile_pool(name="obufp", bufs=4) as obufp,
                ):
                    # HAM warmup: the PE is otherwise idle until the first
                    # input DMAs land (~6us), which guarantees a cold-clock
                    # (K=4/8) start. Throwaway matmuls on a zero tile flip
                    # the activity window to full clock before real work.
                    if rep == 0:
                        dmt = genp.tile([128, 512], f32, tag="g",
                                        name="warm_mm")
                        for _ in range(72):
                            nc.tensor.matmul(dmt[:, 0:128], lhsT=dummy_sb,
                                             rhs=dummy_sb, start=True,
                                             stop=True)
                    def rope_tile(pair, qk, rb):
                        cs = slice(rb * 512, (rb + 1) * 512)
                        t = qkT[(pair, qk, rb)]
                        tmp = ropep.tile([128, 512], bf16, tag="ropetmp",
                                         name="ropetmp")
                        for b0 in (0, 64):
                            nc.vector.tensor_mul(
                                tmp[b0:b0 + 32, :], t[b0 + 32:b0 + 64, :],
                                sin_sb[b0 + 32:b0 + 64, cs])
                            nc.vector.tensor_mul(
                                tmp[b0 + 32:b0 + 64, :], t[b0:b0 + 32, :],
                                sin_sb[b0:b0 + 32, cs])
                        nc.vector.tensor_mul(t[:, :], t[:, :], cos_sb[:, cs])
                        nc.vector.tensor_add(t[:, :], t[:, :], tmp)

                    deferred_ropes = {}

                    def proj_pass_qk(rb, pair, defer=False):
                        ps = []
                        for qk in range(2):
                            ps.append(genp.tile([128, 512], f32, tag="g",
                                                name=f"qk_ps{pair}{qk}"))
                        for kc in range(KC):
                            ht = ht_slice(rb, kc)
                            for qk in range(2):
                                col0 = (pair * 2 + qk) * 128
                                nc.tensor.matmul(
                                    ps[qk],
                                    lhsT=w_sb[:, kc, col0:col0 + 128],
                                    rhs=ht,
                                    start=(kc == 0), stop=(kc == KC - 1))
                        if not defer:
                            # the NEW q gates the next query block's first
                            # score matmuls (kc 0 uses the oldest k): q's
                            # evac + rope jump the ACT/DVE queues
                            with tc.high_priority():
                                nc.scalar.copy(qkT[(pair, 0, rb)], ps[0])
                                rope_tile(pair, 0, rb)
                            with tc.high_priority():
                                nc.scalar.copy(qkT[(pair, 1, rb)], ps[1])
                                rope_tile(pair, 1, rb)
                        else:
                            # pair 1 is needed only at the NEXT pair-1
                            # attention block: evacuate now, rope later as
                            # inline DVE fillers (halves the rope burst
                            # that collides with masks at block entry)
                            for qk in range(2):
                                nc.scalar.copy(qkT[(pair, qk, rb)], ps[qk])
                            deferred_ropes.setdefault(rb, []).extend(
                                (lambda q=qk: rope_tile(pair, q, rb))
                                for qk in range(2))

                    def proj_pass_v(rb, half):
                        rcs = [rb * 4 + half * 2, rb * 4 + half * 2 + 1]
                        ps = []
                        for j in range(2):
                            ps.append(genp.tile([128, 512], f32, tag="g",
                                                name=f"v_ps{j}"))
                        for kc in range(KC):
                            for j, rc in enumerate(rcs):
                                nc.tensor.matmul(
                                    ps[j][:, 0:256],
                                    lhsT=ht_slice(rb, kc)[
                                        :, (rc % 4) * 128:(rc % 4 + 1) * 128],
                                    rhs=w_sb[:, kc, 512:768],
                                    start=(kc == 0), stop=(kc == KC - 1))
                        for j, rc in enumerate(rcs):
                            nc.scalar.copy(
                                vaug[:, rc, :, 0:64],
                                ps[j][:, 0:256].rearrange(
                                    "p (a b) -> p a b", a=4))

                    def proj_rb(rb, defer=False):
                        proj_pass_qk(rb, 0)
                        proj_pass_qk(rb, 1, defer=defer)
                        proj_pass_v(rb, 0)
                        proj_pass_v(rb, 1)

                    LAG = 2

                    def attn_block(qb, pair, fillers=None,
                                   dve_fillers=None):
                        kmax = 4 * qb + 3 if causal else NKC - 1
                        pvs = []
                        for h2 in range(2):
                            pvs.append(pvp.tile([65, 512], f32, tag="pv",
                                                name=f"pv{h2}"))
                        qT = qkT[(pair, 0, qb)]

                        def emit_pv(ent):
                            kc, qlo, psb = ent
                            for h2 in range(2):
                                nc.tensor.matmul(
                                    pvs[h2][:, qlo:],
                                    lhsT=vaug[:, kc, pair * 2 + h2, :],
                                    rhs=psb[:, h2 * 512 + qlo:(h2 + 1) * 512],
                                    start=(kc == 0), stop=(kc == kmax))

                        pending = []
                        for kc in range(kmax + 1):
                            kT = qkT[(pair, 1, kc // 4)]
                            kc4 = kc % 4
                            qlo = (max(0, kc * 128 - qb * 512)
                                   if causal else 0)
                            st = stp.tile([128, 1024], f32, tag="st",
                                          name="st")
                            psb = psbp.tile([128, 1024], bf16, tag="psb",
                                            name="psb")
                            for h2 in range(2):
                                b0 = h2 * 64
                                nc.tensor.matmul(
                                    st[:, h2 * 512 + qlo:(h2 + 1) * 512],
                                    lhsT=kT[b0:b0 + 64,
                                            kc4 * 128:(kc4 + 1) * 128],
                                    rhs=qT[b0:b0 + 64, qlo:512],
                                    start=True, stop=True)
                            if qlo == 0:
                                nc.scalar.activation(psb[:, :], st[:, :],
                                                     func=Exp, scale=0.125)
                            else:
                                st3 = st.rearrange("p (h q) -> p h q", h=2)
                                psb3 = psb.rearrange("p (h q) -> p h q", h=2)
                                nc.scalar.activation(
                                    psb3[:, :, qlo:], st3[:, :, qlo:],
                                    func=Exp, scale=0.125)
                            if causal and kc >= 4 * qb:
                                psbm = psb.rearrange(
                                    "p (h q) -> p h q", h=2)[:, :,
                                                            qlo:qlo + 128]
                                nc.vector.tensor_mul(
                                    psbm, psbm,
                                    tri_sb.rearrange("p (h q) -> p h q", h=2))
                            pending.append((kc, qlo, psb))
                            if len(pending) > LAG:
                                emit_pv(pending.pop(0))
                            if fillers and kc % 2 == 0:
                                fillers.pop(0)()
                            if dve_fillers and kc % 2 == 1:
                                dve_fillers.pop(0)()
                        while pending:
                            emit_pv(pending.pop(0))

                        # normalize: evacuate pv psum fast, fast-approx
                        # reciprocal of the ones-row, broadcast on gpsimd,
                        # multiply into A_T on DVE. High priority: the pv
                        # slot release and A_T availability gate the next
                        # block's PV matmuls and the outproj, and these ops
                        # otherwise queue behind the diagonal-chunk masks.
                        # The LAST block takes a latency-optimized path: the
                        # denominator is read straight from pv psum, the
                        # broadcast runs as two rank-1 matmuls on the (idle)
                        # PE into the freed pv banks, and the pvc copies
                        # overlap the reciprocal.
                        last = (qb == NQB - 1 and pair == 1)
                        with tc.high_priority():
                            den = smallp.tile([1, 1024], f32, tag="den",
                                              name="den")
                            if last:
                                for h2 in range(2):
                                    nc.vector.tensor_copy(
                                        den[:, h2 * 512:(h2 + 1) * 512],
                                        pvs[h2][64:65, :])
                            rinv = smallp.tile([1, 1024], f32, tag="rinv",
                                               name="rinv")
                            if last:
                                # keep the PE clock warm across the final
                                # normalize chain: these become ready only
                                # once `den` is written (i.e. in the gap)
                                dmt2 = genp.tile([128, 512], f32, tag="g",
                                                 name="warm_mm2")
                                for _ in range(6):
                                    nc.tensor.matmul(
                                        dmt2, lhsT=den[0:1, 0:128],
                                        rhs=den[0:1, 0:512], start=True,
                                        stop=True)
                                nc.vector.reciprocal_approx_fast(rinv, den)
                                rinvb = smallp.tile([1, 1024], bf16,
                                                    tag="rinvb", name="rinvb")
                                nc.vector.tensor_copy(rinvb, rinv)
                            pvc = pvcp.tile([65, 1024], f32, tag="pvc",
                                            name="pvc")
                            for h2 in range(2):
                                nc.vector.tensor_copy(
                                    pvc[:, h2 * 512:(h2 + 1) * 512], pvs[h2])
                            if not last:
                                nc.vector.tensor_copy(den, pvc[64:65, :])
                                nc.vector.reciprocal_approx_fast(rinv, den)
                            if last:
                                bc_ps = []
                                for h2 in range(2):
                                    bp = pvp.tile([64, 512], f32, tag="pv",
                                                  name=f"bcps{h2}")
                                    nc.tensor.matmul(
                                        bp, lhsT=ones_col,
                                        rhs=rinvb[:,
                                                  h2 * 512:(h2 + 1) * 512],
                                        start=True, stop=True)
                                    bc_ps.append(bp)
                                for h2 in range(2):
                                    nc.vector.tensor_mul(
                                        A_T[pair][h2 * 64:(h2 + 1) * 64,
                                                  qb * 512:(qb + 1) * 512],
                                        pvc[0:64, h2 * 512:(h2 + 1) * 512],
                                        bc_ps[h2])
                            else:
                                bcast = smallp.tile([64, 1024], f32, tag="bc",
                                                    name="bc")
                                nc.gpsimd.partition_broadcast(bcast, rinv)
                                for h2 in range(2):
                                    nc.vector.tensor_mul(
                                        A_T[pair][h2 * 64:(h2 + 1) * 64,
                                                  qb * 512:(qb + 1) * 512],
                                        pvc[0:64, h2 * 512:(h2 + 1) * 512],
                                        bcast[:, h2 * 512:(h2 + 1) * 512])

                    def outproj(qb):
                        # for the last block, borrow the (now free) score
                        # psum banks so the pair-0 matmuls of several row
                        # chunks can pre-run while the final normalize chain
                        # still computes pair 1's A_T
                        last = (qb == NQB - 1)
                        for rc in range(4 * qb, 4 * qb + 4):
                            ob = obufp.tile([128, 1024], bf16, tag="ob",
                                            name="ob")
                            if last and rc % 4 < 2:
                                opw = stp.tile([128, 1024], f32, tag="st",
                                               name="opw")
                                halves = [opw[:, 0:512], opw[:, 512:1024]]
                            else:
                                halves = None
                            for half in range(2):
                                if halves is None:
                                    op = genp.tile([128, 512], f32, tag="g",
                                                   name="op")
                                else:
                                    op = halves[half]
                                for pair in range(2):
                                    nc.tensor.matmul(
                                        op,
                                        lhsT=A_T[pair][:,
                                                       rc * 128:(rc + 1) * 128],
                                        rhs=wout_sb[:, pair,
                                                    half * 512:(half + 1) * 512],
                                        start=(pair == 0), stop=(pair == 1))
                                if (rc + half) % 2 == 0:
                                    nc.scalar.copy(
                                        ob[:, half * 512:(half + 1) * 512], op)
                                else:
                                    nc.vector.tensor_copy(
                                        ob[:, half * 512:(half + 1) * 512], op)
                            nc.sync.dma_start(
                                out=out_d[rc * 128:(rc + 1) * 128, :], in_=ob)

                    # ---- main emission stream ----
                    ob_map = {}

                    def op_filler(rc, half):
                        def f():
                            if half == 0:
                                ob_map[rc] = obufp.tile(
                                    [128, 1024], bf16, tag="ob", name="ob")
                            ob = ob_map[rc]
                            op = genp.tile([128, 512], f32, tag="g",
                                           name="op")
                            for pair in range(2):
                                nc.tensor.matmul(
                                    op,
                                    lhsT=A_T[pair][:,
                                                   rc * 128:(rc + 1) * 128],
                                    rhs=wout_sb[:, pair,
                                                half * 512:(half + 1) * 512],
                                    start=(pair == 0), stop=(pair == 1))
                            if (rc + half) % 2 == 0:
                                nc.scalar.copy(
                                    ob[:, half * 512:(half + 1) * 512], op)
                            else:
                                nc.vector.tensor_copy(
                                    ob[:, half * 512:(half + 1) * 512], op)
                            if half == 1:
                                nc.sync.dma_start(
                                    out=out_d[rc * 128:(rc + 1) * 128, :],
                                    in_=ob_map.pop(rc))
                        return f

                    proj_rb(0)
                    attn_block(0, 0)
                    proj_rb(1, defer=True)
                    attn_block(0, 1)
                    outproj(0)
                    attn_block(1, 0, dve_fillers=deferred_ropes.get(1))
                    proj_rb(2, defer=True)
                    attn_block(1, 1)
                    attn_block(2, 0, dve_fillers=deferred_ropes.get(2))
                    proj_rb(3, defer=True)
                    attn_block(2, 1)
                    attn_block(3, 0, dve_fillers=deferred_ropes.get(3))
                    outproj(2)
                    # outproj(1) is parked as inline fillers between the
                    # last pair-block's chunks: real PE work that keeps the
                    # HAM clock up through the exp-paced final stretch
                    attn_block(3, 1, fillers=[
                        op_filler(rc, h) for rc in range(4, 8)
                        for h in range(2)])
                    # psum-free HAM filler: standalone dummy weight loads
                    # keep the PE array active through exp-paced stretches
                    # (every real matmul self-loads its weights, so these
                    # cannot corrupt results)
                    for _ in range(60):
                        nc.tensor.ldweights(dummy_sb)
                    outproj(3)
                    for _ in range(40):
                        nc.tensor.ldweights(dummy_sb)

    nc.compile()
    return nc


def _host_inputs(hidden_states, W_qkv, W_out):
    """Build the 8 per-core input maps."""
    import ml_dtypes
    bf16 = ml_dtypes.bfloat16
    hidden = np.ascontiguousarray(hidden_states, dtype=np.float32)
    W_qkv = np.asarray(W_qkv, dtype=np.float32)
    W_out = np.asarray(W_out, dtype=np.float32)
    Wq, Wk, Wv = W_qkv[:, :1024], W_qkv[:, 1024:2048], W_qkv[:, 2048:]

    perm = np.concatenate([np.arange(0, 64, 2), np.arange(1, 64, 2)])

    invf = THETA ** (-np.arange(0, 32, dtype=np.float64) * 2.0 / 64.0)
    ang = np.arange(N, dtype=np.float64)[:, None] * invf[None, :]  # [N, 32]
    jj = np.arange(64)
    cos64 = np.cos(ang)[:, jj % 32].T
    sin64 = np.sin(ang)[:, jj % 32].T
    # row r holds the sin factor applied when row r is the SOURCE of the
    # half-swap: rows j<32 feed dst j+32 with +sin; rows j>=32 feed dst j-32
    # with -sin.
    sgn = np.where(jj < 32, 1.0, -1.0)[:, None]
    cos_t = np.ascontiguousarray(np.tile(cos64, (2, 1)), dtype=bf16)
    sin_t = np.ascontiguousarray(np.tile(sgn * sin64, (2, 1)), dtype=bf16)
    # multiplicative mask: 1 where q >= k (valid), 0 where masked
    tri1 = np.where(
        np.arange(128)[None, :] >= np.arange(128)[:, None], 1.0, 0.0)
    tri = np.ascontiguousarray(
        np.concatenate([tri1, tri1], axis=1), dtype=bf16)

    hT = [np.ascontiguousarray(hidden[b].T.astype(bf16)) for b in range(B)]

    in_maps = []
    for c in range(NCORES):
        bb = c // 4
        bh = (c % 4) * 4

        def qk_cols(pair, qk):
            W = Wq if qk == 0 else Wk
            cols = []
            for h2 in range(2):
                hh = bh + pair * 2 + h2
                cols.extend(hh * 64 + perm)
            return W[:, np.array(cols)]

        w_all = np.ascontiguousarray(np.concatenate(
            [qk_cols(0, 0), qk_cols(0, 1), qk_cols(1, 0), qk_cols(1, 1),
             Wv[:, bh * 64:(bh + 4) * 64]], axis=1), dtype=bf16)
        wout_c = np.ascontiguousarray(W_out[bh * 64:(bh + 4) * 64, :], dtype=bf16)
        in_maps.append({
            "hidden_T": hT[bb],
            "w_all": w_all,
            "w_out": wout_c,
            "cos_t": cos_t,
            "sin_t": sin_t,
            "tri": tri,
        })
    return in_maps


def _unshard(results, b_out):
    out = np.zeros((B, N, 1024), dtype=np.float32)
    for c in range(NCORES):
        out[c // 4] += np.asarray(results[c]["out_partial"], dtype=np.float32)
    out += np.asarray(b_out, dtype=np.float32)[None, None, :]
    return out


def run(hidden_states, W_qkv, W_out, b_out, is_causal, **_ignored):
    causal = bool(int(np.asarray(is_causal)))
    key = ("nc", causal, 1)
    if key not in _compiled:
        _compiled[key] = _build_nc(causal)
    nc = _compiled[key]

    from concourse import bass2jax
    in_maps = _host_inputs(hidden_states, W_qkv, W_out)
    results = bass2jax.run_bass_via_pjrt(nc, in_maps, n_cores=NCORES)
    return _unshard(results, b_out), None


def profile_exec(hidden_states, W_qkv, W_out, b_out, is_causal,
                 neff_dir="/tmp/kprof", all_cores=False):
    """Run once under the axon NTFF profiling hook; return
    (output, exec_time_ns, trace_paths). exec_time_ns is the max profiled
    per-core NEFF execution span (neuron-profile ground truth)."""
    import contextlib
    import ctypes
    import os
    import shutil
    import sys as _sys

    causal = bool(int(np.asarray(is_causal)))
    key = ("nc", causal, 1)
    if key not in _compiled:
        _compiled[key] = _build_nc(causal)
    nc = _compiled[key]
    in_maps = _host_inputs(hidden_states, W_qkv, W_out)

    @contextlib.contextmanager
    def ntff_profile(output_dir, device_ids):
        import jax
        lib = ctypes.CDLL("/opt/axon/libaxon_pjrt.so")
        lib.axon_start_nrt_profile.argtypes = [
            ctypes.POINTER(ctypes.c_int64), ctypes.c_size_t]
        lib.axon_start_nrt_profile.restype = ctypes.c_int64
        lib.axon_stop_nrt_profile.argtypes = [ctypes.c_char_p]
        lib.axon_stop_nrt_profile.restype = ctypes.c_int64
        jax.devices()
        ids = (ctypes.c_int64 * len(device_ids))(*device_ids)
        rc = lib.axon_start_nrt_profile(ids, len(device_ids))
        if rc != 0:
            raise RuntimeError(f"axon_start_nrt_profile rc={rc}")
        try:
            yield
        finally:
            n = lib.axon_stop_nrt_profile(str(output_dir).encode())
            print(f"profile: {n} file(s) written to {output_dir}",
                  file=_sys.stderr)

    shutil.rmtree(neff_dir, ignore_errors=True)
    os.makedirs(neff_dir, exist_ok=True)
    from concourse import bass2jax
    model_indices = list(range(NCORES)) if all_cores else [0]
    with ntff_profile(neff_dir, model_indices):
        results = bass2jax.run_bass_via_pjrt(nc, in_maps, n_cores=NCORES)

    import gauge.profiler
    from concourse._compat import FishPath
    profile = gauge.profiler.Profile(
        profile_path=FishPath(neff_dir),
        kernel_dev_mode=True,
        profile_on_exit=False,
        bass_kernel=nc.m,
        offline_processing=True,
        fname="*_body*",
    )
    perfetto_results = profile.to_perfetto(model_index=tuple(model_indices))
    exec_ns = max(pr.exec_time_ns for pr in perfetto_results)
    traces = [pr.trace_path for pr in perfetto_results]
    return _unshard(results, b_out), exec_ns, traces


def kernel(hidden_states, W_qkv, W_out, b_out, is_causal):
    out, _ = run(hidden_states, W_qkv, W_out, b_out, is_causal)
    return out


# revision 60
# speedup vs baseline: 1.2937x; 1.0517x over previous
"""Trainium2 Bass kernel for multi-head causal attention with rotary embeddings.

Problem shapes (hardcoded):
  hidden_states [2, 2048, 1024] f32, W_qkv [1024, 3072], W_out [1024, 1024],
  b_out [1024], is_causal scalar. 16 heads x 64 dim, rope theta 10000.

Sharding over 8 cores: core c -> batch c//4, heads 4*(c%4) .. 4*(c%4)+3
(data parallel over batch x tensor parallel over heads; W_qkv column-parallel,
W_out row-parallel; per-core partial outputs are summed on host).

Rope trick: head-dim columns of Wq/Wk are de-interleaved on the host
(pairs (2i, 2i+1) -> (i, i+32)) so on-chip rope is a contiguous half-swap;
scores are invariant because q and k share the permutation.

Schedule design (v4, tuned against NTFF hardware profiles; ~180us/NEFF
fast-power-mode vs 308us for the previous filler-based design):
  - One linear emission stream: proj rb0 | attn(qb,p0) | proj rb_{qb+1} |
    attn(qb,p1) | outproj qb | ... The Tile greedy scheduler backfills PE
    idle slots in ACT-bound attention stretches with the next projection
    block's matmuls (no hand-rolled filler machinery).
  - The 1/8 score scale is folded into the exp activation's scale parameter
    (free on ACT) instead of a separate scaled psum evacuation.
  - Softmax denominators ride as an appended ones-row of V (PV row 64);
    reciprocal uses reciprocal_approx_fast (~5x cheaper than exact; input
    must sit at partition 0 - the custom op misreads other bases),
    partition-broadcast on gpsimd, normalize multiplies on DVE, the whole
    chain at high priority (it gates pv-slot reuse and outproj).
  - The score h2 pairs run concurrently on the PE (row groups 0-1 / 2-3
    via base-partition-derived tile_position); projections are emitted in
    2-tile passes so the shared 2-bank psum pool ping-pongs.
  - q/k evac + rope run at high priority (q first: the NEW q gates the
    next query block's first score matmuls): the ACT-copy -> DVE-rope chain
    otherwise queues behind diagonal-chunk masks and stalls the next query
    block's score matmuls.
  - PSUM budget: scores 2x[128,1024] (4 banks) + PV 2x[65,512] (2 banks) +
    shared proj/outproj pool 2x[128,512] (2 banks) = 8 banks exactly. The
    last block's outproj borrows freed score banks; its normalize takes a
    latency path (denominator read from pv psum, broadcast as two rank-1
    PE matmuls into freed pv banks).
  - HAM warmup: 72 throwaway matmuls cover the initial DMA ramp and the
    final normalize gap so real work runs at the 2.4 GHz (K=8/8) clock;
    outproj(1) is parked as inline fillers between the last pair-block's
    chunks to keep PE duty (and the clock) up through the exp-paced tail.
  - PV lags scores by 2 chunks; the multiplicative 0/1 causal mask runs on
    DVE after exp as one [128,2,128] op against a doubled tri table.
  - fp8/DoubleRow was tried and reverted: e4m3 quantization of h/W alone
    costs 6.6% rel error (random-sign sums do not average it out) vs the
    2e-2 gate.
"""

import numpy as np

B, N, D = 2, 2048, 1024
H, DH = 16, 64
THETA = 10000.0
NCORES = 8
KC = D // 128        # 8 contraction chunks for the projections
NKC = N // 128       # 16 key chunks
NQB = N // 512       # 4 query blocks
NRB = N // 512       # 4 row blocks for projections

_compiled = {}


def _build_nc(causal: bool, reps: int = 1):
    import concourse.bass as bass
    import concourse.tile as tile
    from concourse import bacc, mybir

    f32 = mybir.dt.float32
    bf16 = mybir.dt.bfloat16
    Exp = mybir.ActivationFunctionType.Exp

    nc = bacc.Bacc("TRN2", target_bir_lowering=False)
    hT_d = nc.dram_tensor("hidden_T", [D, N], bf16, kind="ExternalInput")
    w_d = nc.dram_tensor("w_all", [D, 768], bf16, kind="ExternalInput")
    wout_d = nc.dram_tensor("w_out", [256, 1024], bf16, kind="ExternalInput")
    cos_d = nc.dram_tensor("cos_t", [128, N], bf16, kind="ExternalInput")
    sin_d = nc.dram_tensor("sin_t", [128, N], bf16, kind="ExternalInput")
    tri_d = nc.dram_tensor("tri", [128, 256], bf16, kind="ExternalInput")
    out_d = nc.dram_tensor("out_partial", [N, 1024], bf16, kind="ExternalOutput")

    w_r = w_d.rearrange("(a p) c -> p a c", p=128)
    wout_r = wout_d.rearrange("(a p) c -> p a c", p=128)

    with tile.TileContext(nc) as tc:
        with (
            tc.tile_pool(name="consts", bufs=1) as consts,
            tc.tile_pool(name="qk", bufs=1) as qkp,
            tc.tile_pool(name="vaugp", bufs=1) as vaugp,
            tc.tile_pool(name="atp", bufs=1) as atp,
            tc.tile_pool(name="htp", bufs=1) as htp,
        ):
            w_sb = consts.tile([128, KC, 768], bf16, tag="w_sb", name="w_sb")
            wout_sb = consts.tile([128, 2, 1024], bf16, tag="wout_sb",
                                  name="wout_sb")
            cos_sb = consts.tile([128, N], bf16, tag="cos_sb", name="cos_sb")
            sin_sb = consts.tile([128, N], bf16, tag="sin_sb", name="sin_sb")
            tri_sb = consts.tile([128, 256], bf16, tag="tri_sb", name="tri_sb")

            ones_col = consts.tile([1, 64], bf16, tag="ones_col",
                                   name="ones_col")
            nc.vector.memset(ones_col, 1.0)
            dummy_sb = consts.tile([128, 128], bf16, tag="dummy_sb",
                                   name="dummy_sb")
            nc.vector.memset(dummy_sb, 0.0)

            # preload the Exp activation table while the first DMAs land
            warm_in = consts.tile([1, 1], f32, tag="warm_in", name="warm_in")
            warm_out = consts.tile([1, 1], f32, tag="warm_out", name="warm_out")
            nc.vector.memset(warm_in, 0.0)
            nc.scalar.activation(warm_out, warm_in, func=Exp)

            for rep in range(reps):
                qkT = {}
                for pair in range(2):
                    for qk in range(2):
                        for rb in range(NRB):
                            t = qkp.tile([128, 512], bf16,
                                         tag=f"qkT{pair}{qk}{rb}",
                                         name=f"qkT{pair}{qk}{rb}")
                            qkT[(pair, qk, rb)] = t
                vaug = vaugp.tile([128, NKC, 4, 65], bf16, tag="vaug",
                                  name="vaug")
                nc.vector.memset(vaug[:, :, :, 64:65], 1.0)
                A_T = {}
                for pair in range(2):
                    A_T[pair] = atp.tile([128, N], bf16, tag=f"AT{pair}",
                                         name=f"AT{pair}")

                htA = {}
                htB = {}
                for kc in range(KC):
                    htA[kc] = htp.tile([128, 1024], bf16, tag=f"htA_{kc}",
                                       name=f"htA_{kc}")
                    htB[kc] = htp.tile([128, 1024], bf16, tag=f"htB_{kc}",
                                       name=f"htB_{kc}")

                def ht_slice(rb, kc):
                    if rb < 2:
                        return htA[kc][:, rb * 512:(rb + 1) * 512]
                    return htB[kc][:, (rb - 2) * 512:(rb - 1) * 512]

                # ---- DMA plan ----
                for kc in range(KC):
                    if rep == 0:
                        nc.sync.dma_start(out=w_sb[:, kc, :], in_=w_r[:, kc, :])
                    nc.sync.dma_start(
                        out=htA[kc], in_=hT_d[kc * 128:(kc + 1) * 128, 0:1024])
                if rep == 0:
                    nc.sync.dma_start(out=cos_sb, in_=cos_d[:, :])
                    nc.sync.dma_start(out=sin_sb, in_=sin_d[:, :])
                    nc.sync.dma_start(out=tri_sb, in_=tri_d[:, :])
                for kc in range(KC):
                    nc.sync.dma_start(
                        out=htB[kc],
                        in_=hT_d[kc * 128:(kc + 1) * 128, 1024:2048])
                if rep == 0:
                    nc.sync.dma_start(out=wout_sb, in_=wout_r)

                with (
                    tc.tile_pool(name="stp", bufs=2, space="PSUM") as stp,
                    tc.tile_pool(name="pvp", bufs=2, space="PSUM") as pvp,
                    tc.tile_pool(name="genp", bufs=2, space="PSUM") as genp,
                    tc.tile_pool(name="psbp", bufs=8) as psbp,
                    tc.tile_pool(name="pvcp", bufs=2) as pvcp,
                    tc.tile_pool(name="smallp", bufs=4) as smallp,
                    tc.tile_pool(name="ropep", bufs=2) as ropep,
                    tc.tile_pool(name="obufp", bufs=4) as obufp,
                ):
                    # HAM warmup: the PE is otherwise idle until the first
                    # input DMAs land (~6us), which guarantees a cold-clock
                    # (K=4/8) start. Throwaway matmuls on a zero tile flip
                    # the activity window to full clock before real work.
                    if rep == 0:
                        dmt = genp.tile([128, 512], f32, tag="g",
                                        name="warm_mm")
                        for _ in range(72):
                            nc.tensor.matmul(dmt[:, 0:128], lhsT=dummy_sb,
                                             rhs=dummy_sb, start=True,
                                             stop=True)
                    def rope_tile(pair, qk, rb):
                        cs = slice(rb * 512, (rb + 1) * 512)
                        t = qkT[(pair, qk, rb)]
                        tmp = ropep.tile([128, 512], bf16, tag="ropetmp",
                                         name="ropetmp")
                        for b0 in (0, 64):
                            nc.vector.tensor_mul(
                                tmp[b0:b0 + 32, :], t[b0 + 32:b0 + 64, :],
                                sin_sb[b0 + 32:b0 + 64, cs])
                            nc.vector.tensor_mul(
                                tmp[b0 + 32:b0 + 64, :], t[b0:b0 + 32, :],
                                sin_sb[b0:b0 + 32, cs])
                        nc.vector.tensor_mul(t[:, :], t[:, :], cos_sb[:, cs])
                        nc.vector.tensor_add(t[:, :], t[:, :], tmp)

                    def proj_pass_qk(rb, pair):
                        ps = []
                        for qk in range(2):
                            ps.append(genp.tile([128, 512], f32, tag="g",
                                                name=f"qk_ps{pair}{qk}"))
                        for kc in range(KC):
                            ht = ht_slice(rb, kc)
                            for qk in range(2):
                                col0 = (pair * 2 + qk) * 128
                                nc.tensor.matmul(
                                    ps[qk],
                                    lhsT=w_sb[:, kc, col0:col0 + 128],
                                    rhs=ht,
                                    start=(kc == 0), stop=(kc == KC - 1))
                        # the NEW q gates the next query block's first score
                        # matmuls (kc 0 uses the oldest k), so q's evac +
                        # rope jump ahead of nearby masks on the ACT/DVE
                        # queues; relative offset (not absolute 0) so the
                        # normalize chain still outranks them at block ends
                        with tc.high_priority():
                            nc.scalar.copy(qkT[(pair, 0, rb)], ps[0])
                            rope_tile(pair, 0, rb)
                        with tc.high_priority():
                            nc.scalar.copy(qkT[(pair, 1, rb)], ps[1])
                            rope_tile(pair, 1, rb)

                    def proj_pass_v(rb, half):
                        rcs = [rb * 4 + half * 2, rb * 4 + half * 2 + 1]
                        ps = []
                        for j in range(2):
                            ps.append(genp.tile([128, 512], f32, tag="g",
                                                name=f"v_ps{j}"))
                        for kc in range(KC):
                            for j, rc in enumerate(rcs):
                                nc.tensor.matmul(
                                    ps[j][:, 0:256],
                                    lhsT=ht_slice(rb, kc)[
                                        :, (rc % 4) * 128:(rc % 4 + 1) * 128],
                                    rhs=w_sb[:, kc, 512:768],
                                    start=(kc == 0), stop=(kc == KC - 1))
                        for j, rc in enumerate(rcs):
                            nc.scalar.copy(
                                vaug[:, rc, :, 0:64],
                                ps[j][:, 0:256].rearrange(
                                    "p (a b) -> p a b", a=4))

                    def proj_rb(rb):
                        proj_pass_qk(rb, 0)
                        proj_pass_qk(rb, 1)
                        proj_pass_v(rb, 0)
                        proj_pass_v(rb, 1)

                    LAG = 2

                    def attn_block(qb, pair, fillers=None):
                        kmax = 4 * qb + 3 if causal else NKC - 1
                        pvs = []
                        for h2 in range(2):
                            pvs.append(pvp.tile([65, 512], f32, tag="pv",
                                                name=f"pv{h2}"))
                        qT = qkT[(pair, 0, qb)]

                        def emit_pv(ent):
                            kc, qlo, psb = ent
                            for h2 in range(2):
                                nc.tensor.matmul(
                                    pvs[h2][:, qlo:],
                                    lhsT=vaug[:, kc, pair * 2 + h2, :],
                                    rhs=psb[:, h2 * 512 + qlo:(h2 + 1) * 512],
                                    start=(kc == 0), stop=(kc == kmax))

                        pending = []
                        for kc in range(kmax + 1):
                            kT = qkT[(pair, 1, kc // 4)]
                            kc4 = kc % 4
                            qlo = (max(0, kc * 128 - qb * 512)
                                   if causal else 0)
                            st = stp.tile([128, 1024], f32, tag="st",
                                          name="st")
                            psb = psbp.tile([128, 1024], bf16, tag="psb",
                                            name="psb")
                            for h2 in range(2):
                                b0 = h2 * 64
                                nc.tensor.matmul(
                                    st[:, h2 * 512 + qlo:(h2 + 1) * 512],
                                    lhsT=kT[b0:b0 + 64,
                                            kc4 * 128:(kc4 + 1) * 128],
                                    rhs=qT[b0:b0 + 64, qlo:512],
                                    start=True, stop=True)
                            if qlo == 0:
                                nc.scalar.activation(psb[:, :], st[:, :],
                                                     func=Exp, scale=0.125)
                            else:
                                st3 = st.rearrange("p (h q) -> p h q", h=2)
                                psb3 = psb.rearrange("p (h q) -> p h q", h=2)
                                nc.scalar.activation(
                                    psb3[:, :, qlo:], st3[:, :, qlo:],
                                    func=Exp, scale=0.125)
                            if causal and kc >= 4 * qb:
                                psbm = psb.rearrange(
                                    "p (h q) -> p h q", h=2)[:, :,
                                                            qlo:qlo + 128]
                                nc.vector.tensor_mul(
                                    psbm, psbm,
                                    tri_sb.rearrange("p (h q) -> p h q", h=2))
                            pending.append((kc, qlo, psb))
                            if len(pending) > LAG:
                                emit_pv(pending.pop(0))
                            if fillers and kc % 2 == 0:
                                fillers.pop(0)()
                        while pending:
                            emit_pv(pending.pop(0))

                        # normalize: evacuate pv psum fast, fast-approx
                        # reciprocal of the ones-row, broadcast on gpsimd,
                        # multiply into A_T on DVE. High priority: the pv
                        # slot release and A_T availability gate the next
                        # block's PV matmuls and the outproj, and these ops
                        # otherwise queue behind the diagonal-chunk masks.
                        # The LAST block takes a latency-optimized path: the
                        # denominator is read straight from pv psum, the
                        # broadcast runs as two rank-1 matmuls on the (idle)
                        # PE into the freed pv banks, and the pvc copies
                        # overlap the reciprocal.
                        last = (qb == NQB - 1 and pair == 1)
                        with tc.high_priority():
                            den = smallp.tile([1, 1024], f32, tag="den",
                                              name="den")
                            if last:
                                for h2 in range(2):
                                    nc.vector.tensor_copy(
                                        den[:, h2 * 512:(h2 + 1) * 512],
                                        pvs[h2][64:65, :])
                            rinv = smallp.tile([1, 1024], f32, tag="rinv",
                                               name="rinv")
                            if last:
                                # keep the PE clock warm across the final
                                # normalize chain: these become ready only
                                # once `den` is written (i.e. in the gap)
                                dmt2 = genp.tile([128, 512], f32, tag="g",
                                                 name="warm_mm2")
                                for _ in range(6):
                                    nc.tensor.matmul(
                                        dmt2, lhsT=den[0:1, 0:128],
                                        rhs=den[0:1, 0:512], start=True,
                                        stop=True)
                                nc.vector.reciprocal_approx_fast(rinv, den)
                                rinvb = smallp.tile([1, 1024], bf16,
                                                    tag="rinvb", name="rinvb")
                                nc.vector.tensor_copy(rinvb, rinv)
                            pvc = pvcp.tile([65, 1024], f32, tag="pvc",
                                            name="pvc")
                            for h2 in range(2):
                                nc.vector.tensor_copy(
                                    pvc[:, h2 * 512:(h2 + 1) * 512], pvs[h2])
                            if not last:
                                nc.vector.tensor_copy(den, pvc[64:65, :])
                                nc.vector.reciprocal_approx_fast(rinv, den)
                            if last:
                                bc_ps = []
                                for h2 in range(2):
                                    bp = pvp.tile([64, 512], f32, tag="pv",
                                                  name=f"bcps{h2}")
                                    nc.tensor.matmul(
                                        bp, lhsT=ones_col,
                                        rhs=rinvb[:,
                                                  h2 * 512:(h2 + 1) * 512],
                                        start=True, stop=True)
                                    bc_ps.append(bp)
                                for h2 in range(2):
                                    nc.vector.tensor_mul(
                                        A_T[pair][h2 * 64:(h2 + 1) * 64,
                                                  qb * 512:(qb + 1) * 512],
                                        pvc[0:64, h2 * 512:(h2 + 1) * 512],
                                        bc_ps[h2])
                            else:
                                bcast = smallp.tile([64, 1024], f32, tag="bc",
                                                    name="bc")
                                nc.gpsimd.partition_broadcast(bcast, rinv)
                                for h2 in range(2):
                                    nc.vector.tensor_mul(
                                        A_T[pair][h2 * 64:(h2 + 1) * 64,
                                                  qb * 512:(qb + 1) * 512],
                                        pvc[0:64, h2 * 512:(h2 + 1) * 512],
                                        bcast[:, h2 * 512:(h2 + 1) * 512])

                    def outproj(qb):
                        # for the last block, borrow the (now free) score
                        # psum banks so the pair-0 matmuls of several row
                        # chunks can pre-run while the final normalize chain
                        # still computes pair 1's A_T
                        last = (qb == NQB - 1)
                        for rc in range(4 * qb, 4 * qb + 4):
                            ob = obufp.tile([128, 1024], bf16, tag="ob",
                                            name="ob")
                            if last and rc % 4 < 2:
                                opw = stp.tile([128, 1024], f32, tag="st",
                                               name="opw")
                                halves = [opw[:, 0:512], opw[:, 512:1024]]
                            else:
                                halves = None
                            for half in range(2):
                                if halves is None:
                                    op = genp.tile([128, 512], f32, tag="g",
                                                   name="op")
                                else:
                                    op = halves[half]
                                for pair in range(2):
                                    nc.tensor.matmul(
                                        op,
                                        lhsT=A_T[pair][:,
                                                       rc * 128:(rc + 1) * 128],
                                        rhs=wout_sb[:, pair,
                                                    half * 512:(half + 1) * 512],
                                        start=(pair == 0), stop=(pair == 1))
                                if (rc + half) % 2 == 0:
                                    nc.scalar.copy(
                                        ob[:, half * 512:(half + 1) * 512], op)
                                else:
                                    nc.vector.tensor_copy(
                                        ob[:, half * 512:(half + 1) * 512], op)
                            nc.sync.dma_start(
                                out=out_d[rc * 128:(rc + 1) * 128, :], in_=ob)

                    # ---- main emission stream ----
                    ob_map = {}

                    def op_filler(rc, half):
                        def f():
                            if half == 0:
                                ob_map[rc] = obufp.tile(
                                    [128, 1024], bf16, tag="ob", name="ob")
                            ob = ob_map[rc]
                            op = genp.tile([128, 512], f32, tag="g",
                                           name="op")
                            for pair in range(2):
                                nc.tensor.matmul(
                                    op,
                                    lhsT=A_T[pair][:,
                                                   rc * 128:(rc + 1) * 128],
                                    rhs=wout_sb[:, pair,
                                                half * 512:(half + 1) * 512],
                                    start=(pair == 0), stop=(pair == 1))
                            if (rc + half) % 2 == 0:
                                nc.scalar.copy(
                                    ob[:, half * 512:(half + 1) * 512], op)
                            else:
                                nc.vector.tensor_copy(
                                    ob[:, half * 512:(half + 1) * 512], op)
                            if half == 1:
                                nc.sync.dma_start(
                                    out=out_d[rc * 128:(rc + 1) * 128, :],
                                    in_=ob_map.pop(rc))
                        return f

                    proj_rb(0)
                    attn_block(0, 0)
                    proj_rb(1)
                    attn_block(0, 1)
                    outproj(0)
                    attn_block(1, 0)
                    proj_rb(2)
                    attn_block(1, 1)
                    attn_block(2, 0)
                    proj_rb(3)
                    attn_block(2, 1)
                    attn_block(3, 0)
                    outproj(2)
                    # outproj(1) is parked as inline fillers between the
                    # last pair-block's chunks: real PE work that keeps the
                    # HAM clock up through the exp-paced final stretch
                    attn_block(3, 1, fillers=[
                        op_filler(rc, h) for rc in range(4, 8)
                        for h in range(2)])
                    # psum-free HAM filler: standalone dummy weight loads
                    # keep the PE array active through exp-paced stretches
                    # (every real matmul self-loads its weights, so these
                    # cannot corrupt results)
                    for _ in range(60):
                        nc.tensor.ldweights(dummy_sb)
                    outproj(3)
                    for _ in range(40):
                        nc.tensor.ldweights(dummy_sb)

    nc.compile()
    return nc


def _host_inputs(hidden_states, W_qkv, W_out):
    """Build the 8 per-core input maps."""
    import ml_dtypes
    bf16 = ml_dtypes.bfloat16
    hidden = np.ascontiguousarray(hidden_states, dtype=np.float32)
    W_qkv = np.asarray(W_qkv, dtype=np.float32)
    W_out = np.asarray(W_out, dtype=np.float32)
    Wq, Wk, Wv = W_qkv[:, :1024], W_qkv[:, 1024:2048], W_qkv[:, 2048:]

    perm = np.concatenate([np.arange(0, 64, 2), np.arange(1, 64, 2)])

    invf = THETA ** (-np.arange(0, 32, dtype=np.float64) * 2.0 / 64.0)
    ang = np.arange(N, dtype=np.float64)[:, None] * invf[None, :]  # [N, 32]
    jj = np.arange(64)
    cos64 = np.cos(ang)[:, jj % 32].T
    sin64 = np.sin(ang)[:, jj % 32].T
    # row r holds the sin factor applied when row r is the SOURCE of the
    # half-swap: rows j<32 feed dst j+32 with +sin; rows j>=32 feed dst j-32
    # with -sin.
    sgn = np.where(jj < 32, 1.0, -1.0)[:, None]
    cos_t = np.ascontiguousarray(np.tile(cos64, (2, 1)), dtype=bf16)
    sin_t = np.ascontiguousarray(np.tile(sgn * sin64, (2, 1)), dtype=bf16)
    # multiplicative mask: 1 where q >= k (valid), 0 where masked
    tri1 = np.where(
        np.arange(128)[None, :] >= np.arange(128)[:, None], 1.0, 0.0)
    tri = np.ascontiguousarray(
        np.concatenate([tri1, tri1], axis=1), dtype=bf16)

    hT = [np.ascontiguousarray(hidden[b].T.astype(bf16)) for b in range(B)]

    in_maps = []
    for c in range(NCORES):
        bb = c // 4
        bh = (c % 4) * 4

        def qk_cols(pair, qk):
            W = Wq if qk == 0 else Wk
            cols = []
            for h2 in range(2):
                hh = bh + pair * 2 + h2
                cols.extend(hh * 64 + perm)
            return W[:, np.array(cols)]

        w_all = np.ascontiguousarray(np.concatenate(
            [qk_cols(0, 0), qk_cols(0, 1), qk_cols(1, 0), qk_cols(1, 1),
             Wv[:, bh * 64:(bh + 4) * 64]], axis=1), dtype=bf16)
        wout_c = np.ascontiguousarray(W_out[bh * 64:(bh + 4) * 64, :], dtype=bf16)
        in_maps.append({
            "hidden_T": hT[bb],
            "w_all": w_all,
            "w_out": wout_c,
            "cos_t": cos_t,
            "sin_t": sin_t,
            "tri": tri,
        })
    return in_maps


def _unshard(results, b_out):
    out = np.zeros((B, N, 1024), dtype=np.float32)
    for c in range(NCORES):
        out[c // 4] += np.asarray(results[c]["out_partial"], dtype=np.float32)
    out += np.asarray(b_out, dtype=np.float32)[None, None, :]
    return out


def run(hidden_states, W_qkv, W_out, b_out, is_causal, **_ignored):
    causal = bool(int(np.asarray(is_causal)))
    key = ("nc", causal, 1)
    if key not in _compiled:
        _compiled[key] = _build_nc(causal)
    nc = _compiled[key]

    from concourse import bass2jax
    in_maps = _host_inputs(hidden_states, W_qkv, W_out)
    results = bass2jax.run_bass_via_pjrt(nc, in_maps, n_cores=NCORES)
    return _unshard(results, b_out), None


def profile_exec(hidden_states, W_qkv, W_out, b_out, is_causal,
                 neff_dir="/tmp/kprof", all_cores=False):
    """Run once under the axon NTFF profiling hook; return
    (output, exec_time_ns, trace_paths). exec_time_ns is the max profiled
    per-core NEFF execution span (neuron-profile ground truth)."""
    import contextlib
    import ctypes
    import os
    import shutil
    import sys as _sys

    causal = bool(int(np.asarray(is_causal)))
    key = ("nc", causal, 1)
    if key not in _compiled:
        _compiled[key] = _build_nc(causal)
    nc = _compiled[key]
    in_maps = _host_inputs(hidden_states, W_qkv, W_out)

    @contextlib.contextmanager
    def ntff_profile(output_dir, device_ids):
        import jax
        lib = ctypes.CDLL("/opt/axon/libaxon_pjrt.so")
        lib.axon_start_nrt_profile.argtypes = [
            ctypes.POINTER(ctypes.c_int64), ctypes.c_size_t]
        lib.axon_start_nrt_profile.restype = ctypes.c_int64
        lib.axon_stop_nrt_profile.argtypes = [ctypes.c_char_p]
        lib.axon_stop_nrt_profile.restype = ctypes.c_int64
        jax.devices()
        ids = (ctypes.c_int64 * len(device_ids))(*device_ids)
        rc = lib.axon_start_nrt_profile(ids, len(device_ids))
        if rc != 0:
            raise RuntimeError(f"axon_start_nrt_profile rc={rc}")
        try:
            yield
        finally:
            n = lib.axon_stop_nrt_profile(str(output_dir).encode())
            print(f"profile: {n} file(s) written to {output_dir}",
                  file=_sys.stderr)

    shutil.rmtree(neff_dir, ignore_errors=True)
    os.makedirs(neff_dir, exist_ok=True)
    from concourse import bass2jax
    model_indices = list(range(NCORES)) if all_cores else [0]
    with ntff_profile(neff_dir, model_indices):
        results = bass2jax.run_bass_via_pjrt(nc, in_maps, n_cores=NCORES)

    import gauge.profiler
    from concourse._compat import FishPath
    profile = gauge.profiler.Profile(
        profile_path=FishPath(neff_dir),
        kernel_dev_mode=True,
        profile_on_exit=False,
        bass_kernel=nc.m,
        offline_processing=True,
        fname="*_body*",
    )
    perfetto_results = profile.to_perfetto(model_index=tuple(model_indices))
    exec_ns = max(pr.exec_time_ns for pr in perfetto_results)
    traces = [pr.trace_path for pr in perfetto_results]
    return _unshard(results, b_out), exec_ns, traces


def kernel(hidden_states, W_qkv, W_out, b_out, is_causal):
    out, _ = run(hidden_states, W_qkv, W_out, b_out, is_causal)
    return out
